# revision 11
# baseline (speedup 1.0000x reference)
"""Multi-head attention kernel for Trainium2 (Bass/Tile), 8 NeuronCores.

Problem: nn_MultiHeadAttention  (B=4, S=2048, D=1024, H=16, DK=64)
    out = softmax((q Wq^T + bq)(k Wk^T + bk)^T / sqrt(DK)) (v Wv^T + bv) Wo^T + bo

Sharding: core c = 2*b + g handles batch b and head-group g (8 heads = 512
features).  Each core computes its batch's attention for its heads plus a
partial output projection; the host sums the two partials per batch.

Math simplifications done on the host (exact):
  - k-bias bk drops out (softmax is shift invariant along the key axis).
  - v-bias bv folds into an effective output bias bo_eff = bo + Wo @ bv.
  - the 1/sqrt(DK) logit scale is folded into Wq/bq.

v3 performance structure (per core):
  - QK^T matmuls run as row-tiled head PAIRS: heads (2p, 2p+1) occupy
    partition halves 0-63 / 64-127 of Q^T/K^T, so their matmuls land on
    disjoint row-halves of the PE array (tile_position (0,0) vs (64,0))
    and execute concurrently -> ~2x on the logit matmuls.
  - exp is split across two engines: ScalarE runs the exact ACT Exp for
    9/16 of tiles, the DVE computes 7/16 with a Schraudolph fast exp
    (int16(x*128/ln2 + (127*128 - C)) bit-cast to bf16).
  - PV, QKV projections and the output projection use N=1024 moving
    operands (half the matmul/LDWEIGHTS count of N=512).
  - pipeline: w0 interleaves the V projection, pair-0 QK+exp, and head-0
    PV; w1..w6 drain one head's PV while producing the next pair's QK at
    half rate; w7 drains head 7; then the output projection.
  - PSUM: 4 banks rotate QK pair tiles (pss, [128,512]), 4 banks hold two
    [*,1024] tiles (pspool): projection psums / PV accumulators.  V
    carries a ones column so the PV matmul emits the softmax denominator
    for free (row 64).
"""

import numpy as np
import ml_dtypes
from contextlib import ExitStack

import concourse.bass as bass
import concourse.tile as tile
from concourse import bacc, mybir
from concourse.bass import ts, ds
from concourse.bass_utils import run_bass_kernel_spmd

B, S, D, H, DK = 4, 2048, 1024, 16, 64
N_CORES = 8
F32 = mybir.dt.float32
BF16 = mybir.dt.bfloat16
I16 = mybir.dt.int16
AF = mybir.ActivationFunctionType
ALU = mybir.AluOpType
BF16NP = ml_dtypes.bfloat16

# Schraudolph fast-exp constants (bf16 bit pattern via int16):
#   E = bitcast_bf16(int16(x * 128/ln2 + (127*128 - C)))
SCH_A = 128.0 / float(np.log(2.0))
SCH_C = 5.8
SCH_B = 127.0 * 128.0 - SCH_C
# fraction of E tiles computed on the DVE (Bresenham NUM/DEN)
DVE_NUM, DVE_DEN = 33, 64


def build_nc(s: int = S):
    """Build + compile the per-core Bass module (SPMD: same NEFF, per-core data)."""
    assert s % 1024 == 0  # x tiles are [128,1024]
    nsi = s // 128   # 128-row key chunks
    nf = s // 512    # 512-col query chunks
    nfp = s // 1024  # 1024-col query pair-chunks

    nc = bacc.Bacc("TRN2", target_bir_lowering=False, debug=False)

    qT = nc.dram_tensor("qT", [D, s], BF16, kind="ExternalInput").ap()
    kT = nc.dram_tensor("kT", [D, s], BF16, kind="ExternalInput").ap()
    vT = nc.dram_tensor("vT", [D, s], BF16, kind="ExternalInput").ap()
    wq = nc.dram_tensor("wq", [D, 512], BF16, kind="ExternalInput").ap()
    wk = nc.dram_tensor("wk", [D, 512], BF16, kind="ExternalInput").ap()
    wv = nc.dram_tensor("wv", [D, 512], BF16, kind="ExternalInput").ap()
    wo = nc.dram_tensor("wo", [512, D], BF16, kind="ExternalInput").ap()
    bq = nc.dram_tensor("bq", [128, 4], F32, kind="ExternalInput").ap()
    outT = nc.dram_tensor("outT", [D, s], F32, kind="ExternalOutput").ap()

    with tile.TileContext(nc) as tc, ExitStack() as ctx:
        pers = ctx.enter_context(tc.tile_pool(name="pers", bufs=1))
        pspool = ctx.enter_context(tc.tile_pool(name="ps", bufs=4, space="PSUM"))
        pss = ctx.enter_context(tc.tile_pool(name="pss", bufs=4, space="PSUM"))
        epool = ctx.enter_context(tc.tile_pool(name="e", bufs=72))

        QT = pers.tile([128, 4, s], BF16)       # Q'^T  [feature, seq]
        KT = pers.tile([128, 4, s], BF16)       # K^T   [feature, seq]
        # V nat [seq, head, dv|ones|pad] + ghost 9th head so PV lhsT can be
        # padded to 128 columns (full-width weights enable fast weight load)
        Vt = pers.tile([128, nsi, 9 * 66], BF16)
        V = Vt.rearrange("p n (h e) -> p n h e", h=9)
        O = pers.tile([128, 4, s], BF16)        # O^T normalized
        WO = pers.tile([128, 4, D], BF16)
        BQ = pers.tile([128, 4], F32)

        nc.sync.dma_start(BQ[:], bq)
        nc.vector.memset(V[:, :, 0:8, 64:66], 1.0)
        nc.vector.memset(V[:, :, 8, :], 0.0)

        # PE warm-up: dependency-free matmuls on a zeroed scratch tile ramp
        # the HAM clock gate while the first input DMAs are still in flight.
        scratch = epool.tile([128, 512], BF16, tag="e", name="warmup_scratch")
        nc.vector.memset(scratch[:], 0.0)
        wps = pss.tile([128, 512], F32, tag="s", name="warmup_ps")
        for _ in range(12):
            nc.tensor.matmul(
                wps[:], lhsT=scratch[:, 0:128], rhs=scratch[:],
                start=True, stop=True, skip_group_check=True,
            )

        ph1 = ExitStack()
        xpool = ph1.enter_context(tc.tile_pool(name="x", bufs=8))
        wpool = ph1.enter_context(tc.tile_pool(name="w", bufs=2))
        ph2b = ExitStack()
        bpool = ph2b.enter_context(tc.tile_pool(name="b", bufs=2))
        oupool = ph2b.enter_context(tc.tile_pool(name="ou", bufs=1))
        dpool = ph2b.enter_context(tc.tile_pool(name="dscr", bufs=4, space="DRAM"))

        dma_engines = [nc.sync, nc.gpsimd]

        # E[h][si][f] -> AP (bf16 view) for the PV matmuls
        E: dict = {h: {} for h in range(8)}
        expc = [0]  # exp tile counter for the engine split

        def emit_exp(h, si, f, ps):
            c = expc[0]
            expc[0] += 1
            use_dve = ((c + 1) * DVE_NUM) // DVE_DEN > (c * DVE_NUM) // DVE_DEN
            if use_dve:
                e = epool.tile([128, 512], I16, tag="e", name=f"e_{h}_{si}_{f}")
                nc.vector.tensor_scalar(
                    e[:], ps[:], SCH_A, SCH_B, ALU.mult, ALU.add
                )
                E[h].setdefault(si, {})[f] = e.bitcast(BF16)
            else:
                e = epool.tile([128, 512], BF16, tag="e", name=f"e_{h}_{si}_{f}")
                nc.scalar.activation(e[:], ps[:], AF.Exp)
                E[h].setdefault(si, {})[f] = e[:]

        def emit_qk(p, si):
            """Row-tiled pair: head 2p on PE rows 0-63, head 2p+1 on 64-127."""
            for f in range(nf):
                psA = pss.tile([128, 512], F32, tag="s", name=f"sA_{p}_{si}_{f}")
                psB = pss.tile([128, 512], F32, tag="s", name=f"sB_{p}_{si}_{f}")
                nc.tensor.matmul(
                    psA[:], lhsT=KT[ds(0, 64), p, ts(si, 128)],
                    rhs=QT[ds(0, 64), p, ts(f, 512)], start=True, stop=True,
                )
                nc.tensor.matmul(
                    psB[:], lhsT=KT[ds(64, 64), p, ts(si, 128)],
                    rhs=QT[ds(64, 64), p, ts(f, 512)], start=True, stop=True,
                )
                emit_exp(2 * p, si, f, psA)
                emit_exp(2 * p + 1, si, f, psB)

        def emit_pv(h, si, pos):
            for f in range(nf):
                nc.tensor.matmul(
                    pos[f][:, :],
                    lhsT=Vt[:, si, ds(h * 66, 128)],
                    rhs=E[h][si].pop(f),
                    start=(si == 0),
                    stop=(si == nsi - 1),
                )

        def pv_finish(h, pos):
            """Per f-block: copy O_unnorm^T + denom out of PSUM (ScalarE,
            frees the accumulator bank), run the denominator chain (DMA
            reshape -> DVE reciprocal -> DMA partition-broadcast) and
            normalize on GpSimd.  f-granular so the last head's chain
            pipelines with the output projection."""
            hp, hh = h // 2, (h % 2) * 64
            ou = oupool.tile([65, s], F32, tag="ou", name=f"ou_{h}")
            for f in range(nf):
                nc.scalar.copy(ou[:, ts(f, 512)], pos[f][0:65, :])
                dscr = dpool.tile([1, 512], F32, tag="dscr", name=f"dscr_{h}_{f}")
                nc.sync.dma_start(dscr[:], ou[ds(64, 1), ts(f, 512)])
                d16 = bpool.tile([16, 32], F32, tag="d16", name=f"d16_{h}_{f}")
                nc.sync.dma_start(
                    d16[:], dscr[:].rearrange("one (p c) -> (one p) c", p=16)
                )
                r16 = bpool.tile([16, 32], F32, tag="r16", name=f"r16_{h}_{f}")
                nc.vector.reciprocal(r16[:], d16[:])
                dsc2 = dpool.tile([1, 512], F32, tag="dsc2", name=f"dsc2_{h}_{f}")
                nc.sync.dma_start(
                    dsc2[:].rearrange("one (p c) -> (one p) c", p=16), r16[:]
                )
                bsb = bpool.tile([64, 512], F32, tag="bsb", name=f"bsb_{h}_{f}")
                nc.sync.dma_start(bsb[:], dsc2[:].to_broadcast((64, 512)))
                eng = nc.gpsimd if f % 2 == 0 else nc.vector
                eng.tensor_tensor(
                    O[ds(hh, 64), hp, ts(f, 512)],
                    ou[0:64, ts(f, 512)],
                    bsb[:],
                    ALU.mult,
                )

        # ---- phase A: Q'/K' projections -------------------------------
        for xdram, wdram, dst, bias in ((qT, wq, QT, BQ), (kT, wk, KT, None)):
            wt = wpool.tile([128, 8, 512], BF16, tag="w")
            for ki in range(8):
                nc.sync.dma_start(wt[:, ki, :], wdram[ds(ki * 128, 128), :])
            for fp in range(nfp):
                xts = []
                for ki in range(8):
                    xt = xpool.tile([128, 1024], BF16, tag="x")
                    dma_engines[ki % 2].dma_start(
                        xt[:], xdram[ds(ki * 128, 128), ds(fp * 1024, 1024)]
                    )
                    xts.append(xt)
                for pc in range(4):
                    for half in range(2):
                        ps = pspool.tile([128, 512], F32, tag="ps")
                        for ki in range(8):
                            nc.tensor.matmul(
                                ps[:],
                                lhsT=wt[:, ki, ts(pc, 128)],
                                rhs=xts[ki][:, ts(half, 512)],
                                start=(ki == 0),
                                stop=(ki == 7),
                            )
                        f = 2 * fp + half
                        if bias is not None:
                            nc.vector.tensor_scalar_add(
                                dst[:, pc, ts(f, 512)], ps[:],
                                bias[:, pc : pc + 1],
                            )
                        else:
                            nc.scalar.copy(dst[:, pc, ts(f, 512)], ps[:])

        # ---- w0: V projection + pair-0 QK/exp + head-0 PV -------------
        wtv = wpool.tile([128, 8, 512], BF16, tag="w")
        for ki in range(8):
            nc.sync.dma_start(wtv[:, ki, :], wv[ds(ki * 128, 128), :])
        pos_cur = [
            pspool.tile([128, 512], F32, tag="ps", name=f"pos_0_{i}")
            for i in range(nf)
        ]
        xv = None
        for si in range(nsi):
            fv, sj = si // 8, si % 8
            if sj == 0:
                xv = []
                for ki in range(8):
                    xt = xpool.tile([128, 1024], BF16, tag="x")
                    dma_engines[ki % 2].dma_start(
                        xt[:], vT[ds(ki * 128, 128), ds(fv * 1024, 1024)]
                    )
                    xv.append(xt)
            vps = pss.tile([128, 512], F32, tag="s", name=f"vps_{si}")
            for ki in range(8):
                nc.tensor.matmul(
                    vps[:],
                    lhsT=xv[ki][:, ts(sj, 128)],
                    rhs=wtv[:, ki, :],
                    start=(ki == 0),
                    stop=(ki == 7),
                )
            nc.scalar.copy(
                V[:, si, 0:8, 0:64], vps[:].rearrange("p (h d) -> p h d", h=8)
            )
            emit_qk(0, si)
            if si > 0:
                emit_pv(0, si - 1, pos_cur)
        emit_pv(0, nsi - 1, pos_cur)
        pv_finish(0, pos_cur)

        for ki in range(4):
            nc.sync.dma_start(WO[:, ki, :], wo[ds(ki * 128, 128), :])

        # ---- w1..w7: drain heads 1..7, produce pairs 1..3 at half rate
        qk_chunks = [(p, si) for p in range(1, 4) for si in range(nsi)]
        qi = 0
        for hd in range(1, 8):
            pos_cur = [
                pspool.tile([128, 512], F32, tag="ps", name=f"pos_{hd}_{i}")
                for i in range(nf)
            ]
            for si in range(nsi):
                emit_pv(hd, si, pos_cur)
                if si % 2 == 0 and qi < len(qk_chunks):
                    emit_qk(*qk_chunks[qi])
                    qi += 1
            if hd == 7:
                # bridge the w7 -> out-proj boundary: these matmuls only
                # need heads 0-5, so they run (from the idle pss banks)
                # while head 7's normalize chain completes, keeping the
                # PE clock warm.
                bridge = []
                for pe in range(4):
                    bps = pss.tile([128, 512], F32, tag="s", name=f"br_{pe}")
                    for ki in range(3):
                        nc.tensor.matmul(
                            bps[:],
                            lhsT=WO[:, ki, ts(pe, 128)],
                            rhs=O[:, ki, ts(0, 512)],
                            start=(ki == 0),
                            stop=False,
                        )
                    bridge.append(bps)
            pv_finish(hd, pos_cur)
        ph2b.close()
        ph1.close()

        # ---- phase C: output projection (partial over this core's heads)
        opool = ctx.enter_context(tc.tile_pool(name="ostage", bufs=3))
        outr = outT.rearrange("(o p) n -> p o n", p=128)
        for pe in range(4):
            nc.tensor.matmul(
                bridge[pe],
                lhsT=WO[:, 3, ts(pe, 128)],
                rhs=O[:, 3, ts(0, 512)],
                start=False,
                stop=True,
            )
            ot = opool.tile([128, 512], F32, tag="ot")
            if pe % 2 == 0:
                nc.vector.tensor_copy(ot[:], bridge[pe])
            else:
                nc.scalar.copy(ot[:], bridge[pe])
            nc.sync.dma_start(outr[:, pe, ts(0, 512)], ot[:])
        for f in range(nf):
            for pe in range(8):
                if f == 0 and pe < 4:
                    continue
                ps = pspool.tile([128, 512], F32, tag="ps")
                for ki in range(4):
                    nc.tensor.matmul(
                        ps[:],
                        lhsT=WO[:, ki, ts(pe, 128)],
                        rhs=O[:, ki, ts(f, 512)],
                        start=(ki == 0),
                        stop=(ki == 3),
                    )
                ot = opool.tile([128, 512], F32, tag="ot")
                if (pe + f) % 2 == 0:
                    nc.vector.tensor_copy(ot[:], ps[:])
                else:
                    nc.scalar.copy(ot[:], ps[:])
                nc.sync.dma_start(outr[:, pe, ts(f, 512)], ot[:])

    nc.compile()
    return nc


_NC_CACHE: dict = {}


def get_nc(s: int = S):
    if s not in _NC_CACHE:
        _NC_CACHE[s] = build_nc(s)
    return _NC_CACHE[s]


def _prep_in_maps(q, k, v, Wq, bq, Wk, Wv, Wo):
    """Host-side shard prep: per-core input dicts (cheap numpy reshapes)."""
    f32 = np.float32
    scale = 1.0 / np.sqrt(DK)
    xT = {}
    for b in range(B):
        xT[b] = (
            np.ascontiguousarray(q[b].T).astype(BF16NP),
            np.ascontiguousarray(k[b].T).astype(BF16NP),
            np.ascontiguousarray(v[b].T).astype(BF16NP),
        )
    per_g = {}
    for g in range(2):
        F = slice(512 * g, 512 * g + 512)
        per_g[g] = dict(
            wq=np.ascontiguousarray(Wq[F].T * scale).astype(BF16NP),
            wk=np.ascontiguousarray(Wk[F].T).astype(BF16NP),
            wv=np.ascontiguousarray(Wv[F].T).astype(BF16NP),
            wo=np.ascontiguousarray(Wo[:, F].T).astype(BF16NP),
            bq=np.ascontiguousarray(
                (bq[F] * scale).reshape(4, 128).T, dtype=f32
            ),
        )
    in_maps = []
    for c in range(N_CORES):
        b, g = c // 2, c % 2
        qb, kb, vb = xT[b]
        in_maps.append(dict(qT=qb, kT=kb, vT=vb, **per_g[g]))
    return in_maps


def kernel(q, k, v, Wq, bq, Wk, bk, Wv, bv, Wo, bo):
    q, k, v = (np.asarray(x, np.float32) for x in (q, k, v))
    Wq, bq, Wk, bk = (np.asarray(x, np.float32) for x in (Wq, bq, Wk, bk))
    Wv, bv, Wo, bo = (np.asarray(x, np.float32) for x in (Wv, bv, Wo, bo))

    nc = get_nc(S)
    in_maps = _prep_in_maps(q, k, v, Wq, bq, Wk, Wv, Wo)
    res = run_bass_kernel_spmd(nc, in_maps, core_ids=list(range(N_CORES)))

    # bk drops out of softmax; bv folds into an effective output bias.
    bo_eff = (
        bo.astype(np.float64) + Wo.astype(np.float64) @ bv.astype(np.float64)
    ).astype(np.float32)
    out = np.empty((B, S, D), np.float32)
    for b in range(B):
        acc = res.results[2 * b]["outT"] + res.results[2 * b + 1]["outT"]
        out[b] = acc.T + bo_eff
    return out


# revision 12
# speedup vs baseline: 1.0508x; 1.0508x over previous
"""Multi-head attention kernel for Trainium2 (Bass/Tile), 8 NeuronCores.

Problem: nn_MultiHeadAttention  (B=4, S=2048, D=1024, H=16, DK=64)
    out = softmax((q Wq^T + bq)(k Wk^T + bk)^T / sqrt(DK)) (v Wv^T + bv) Wo^T + bo

Sharding: core c = 2*b + g handles batch b and head-group g (8 heads = 512
features).  Each core computes its batch's attention for its heads plus a
partial output projection; the host sums the two partials per batch.

Math simplifications done on the host (exact):
  - k-bias bk drops out (softmax is shift invariant along the key axis).
  - v-bias bv folds into an effective output bias bo_eff = bo + Wo @ bv.
  - the 1/sqrt(DK) logit scale is folded into Wq/bq.

v3 performance structure (per core):
  - QK^T matmuls run as row-tiled head PAIRS: heads (2p, 2p+1) occupy
    partition halves 0-63 / 64-127 of Q^T/K^T, so their matmuls land on
    disjoint row-halves of the PE array (tile_position (0,0) vs (64,0))
    and execute concurrently -> ~2x on the logit matmuls.
  - exp is split across two engines: ScalarE runs the exact ACT Exp for
    9/16 of tiles, the DVE computes 7/16 with a Schraudolph fast exp
    (int16(x*128/ln2 + (127*128 - C)) bit-cast to bf16).
  - PV, QKV projections and the output projection use N=1024 moving
    operands (half the matmul/LDWEIGHTS count of N=512).
  - pipeline: w0 interleaves the V projection, pair-0 QK+exp, and head-0
    PV; w1..w6 drain one head's PV while producing the next pair's QK at
    half rate; w7 drains head 7; then the output projection.
  - PSUM: 4 banks rotate QK pair tiles (pss, [128,512]), 4 banks hold two
    [*,1024] tiles (pspool): projection psums / PV accumulators.  V
    carries a ones column so the PV matmul emits the softmax denominator
    for free (row 64).
"""

import numpy as np
import ml_dtypes
from contextlib import ExitStack

import concourse.bass as bass
import concourse.tile as tile
from concourse import bacc, mybir
from concourse.bass import ts, ds
from concourse.bass_utils import run_bass_kernel_spmd

B, S, D, H, DK = 4, 2048, 1024, 16, 64
N_CORES = 8
F32 = mybir.dt.float32
BF16 = mybir.dt.bfloat16
I16 = mybir.dt.int16
AF = mybir.ActivationFunctionType
ALU = mybir.AluOpType
BF16NP = ml_dtypes.bfloat16

# Schraudolph fast-exp constants (bf16 bit pattern via int16):
#   E = bitcast_bf16(int16(x * 128/ln2 + (127*128 - C)))
SCH_A = 128.0 / float(np.log(2.0))
SCH_C = 5.8
SCH_B = 127.0 * 128.0 - SCH_C
# fraction of E tiles computed on the DVE (Bresenham NUM/DEN)
DVE_NUM, DVE_DEN = 33, 64


def build_nc(s: int = S):
    """Build + compile the per-core Bass module (SPMD: same NEFF, per-core data)."""
    assert s % 1024 == 0  # x tiles are [128,1024]
    nsi = s // 128   # 128-row key chunks
    nf = s // 512    # 512-col query chunks
    nfp = s // 1024  # 1024-col query pair-chunks

    nc = bacc.Bacc("TRN2", target_bir_lowering=False, debug=False)

    qT = nc.dram_tensor("qT", [D, s], BF16, kind="ExternalInput").ap()
    kT = nc.dram_tensor("kT", [D, s], BF16, kind="ExternalInput").ap()
    vT = nc.dram_tensor("vT", [D, s], BF16, kind="ExternalInput").ap()
    wq = nc.dram_tensor("wq", [D, 512], BF16, kind="ExternalInput").ap()
    wk = nc.dram_tensor("wk", [D, 512], BF16, kind="ExternalInput").ap()
    wv = nc.dram_tensor("wv", [D, 512], BF16, kind="ExternalInput").ap()
    wo = nc.dram_tensor("wo", [512, D], BF16, kind="ExternalInput").ap()
    bq = nc.dram_tensor("bq", [128, 4], F32, kind="ExternalInput").ap()
    outT = nc.dram_tensor("outT", [D, s], F32, kind="ExternalOutput").ap()

    with tile.TileContext(nc) as tc, ExitStack() as ctx:
        pers = ctx.enter_context(tc.tile_pool(name="pers", bufs=1))
        pspool = ctx.enter_context(tc.tile_pool(name="ps", bufs=4, space="PSUM"))
        pss = ctx.enter_context(tc.tile_pool(name="pss", bufs=4, space="PSUM"))
        epool = ctx.enter_context(tc.tile_pool(name="e", bufs=72))

        QT = pers.tile([128, 4, s], BF16)       # Q'^T  [feature, seq]
        KT = pers.tile([128, 4, s], BF16)       # K^T   [feature, seq]
        # V nat [seq, head, dv|ones|pad] + ghost 9th head so PV lhsT can be
        # padded to 128 columns (full-width weights enable fast weight load)
        Vt = pers.tile([128, nsi, 9 * 66], BF16)
        V = Vt.rearrange("p n (h e) -> p n h e", h=9)
        O = pers.tile([128, 4, s], BF16)        # O^T normalized
        WO = pers.tile([128, 4, D], BF16)
        BQ = pers.tile([128, 4], F32)

        nc.sync.dma_start(BQ[:], bq)
        nc.vector.memset(V[:, :, 0:8, 64:66], 1.0)
        nc.vector.memset(V[:, :, 8, :], 0.0)

        # PE warm-up: dependency-free matmuls on a zeroed scratch tile ramp
        # the HAM clock gate while the first input DMAs are still in flight.
        scratch = epool.tile([128, 512], BF16, tag="e", name="warmup_scratch")
        nc.vector.memset(scratch[:], 0.0)
        wps = pss.tile([128, 512], F32, tag="s", name="warmup_ps")
        for _ in range(12):
            nc.tensor.matmul(
                wps[:], lhsT=scratch[:, 0:128], rhs=scratch[:],
                start=True, stop=True, skip_group_check=True,
            )

        ph1 = ExitStack()
        xpool = ph1.enter_context(tc.tile_pool(name="x", bufs=8))
        wpool = ph1.enter_context(tc.tile_pool(name="w", bufs=2))
        ph2b = ExitStack()
        bpool = ph2b.enter_context(tc.tile_pool(name="b", bufs=2))
        oupool = ph2b.enter_context(tc.tile_pool(name="ou", bufs=1))
        dpool = ph2b.enter_context(tc.tile_pool(name="dscr", bufs=4, space="DRAM"))

        dma_engines = [nc.sync, nc.sync]

        # E[h][si][f] -> AP (bf16 view) for the PV matmuls
        E: dict = {h: {} for h in range(8)}
        expc = [0]  # exp tile counter for the engine split

        def emit_exp(h, si, f, ps):
            c = expc[0]
            expc[0] += 1
            use_dve = ((c + 1) * DVE_NUM) // DVE_DEN > (c * DVE_NUM) // DVE_DEN
            if use_dve:
                e = epool.tile([128, 512], I16, tag="e", name=f"e_{h}_{si}_{f}")
                nc.vector.tensor_scalar(
                    e[:], ps[:], SCH_A, SCH_B, ALU.mult, ALU.add
                )
                E[h].setdefault(si, {})[f] = e.bitcast(BF16)
            else:
                e = epool.tile([128, 512], BF16, tag="e", name=f"e_{h}_{si}_{f}")
                nc.scalar.activation(e[:], ps[:], AF.Exp)
                E[h].setdefault(si, {})[f] = e[:]

        def emit_qk(p, si):
            """Row-tiled pair: head 2p on PE rows 0-63, head 2p+1 on 64-127."""
            for f in range(nf):
                psA = pss.tile([128, 512], F32, tag="s", name=f"sA_{p}_{si}_{f}")
                psB = pss.tile([128, 512], F32, tag="s", name=f"sB_{p}_{si}_{f}")
                nc.tensor.matmul(
                    psA[:], lhsT=KT[ds(0, 64), p, ts(si, 128)],
                    rhs=QT[ds(0, 64), p, ts(f, 512)], start=True, stop=True,
                )
                nc.tensor.matmul(
                    psB[:], lhsT=KT[ds(64, 64), p, ts(si, 128)],
                    rhs=QT[ds(64, 64), p, ts(f, 512)], start=True, stop=True,
                )
                emit_exp(2 * p, si, f, psA)
                emit_exp(2 * p + 1, si, f, psB)

        def emit_pv(h, si, pos):
            for f in range(nf):
                nc.tensor.matmul(
                    pos[f][:, :],
                    lhsT=Vt[:, si, ds(h * 66, 128)],
                    rhs=E[h][si].pop(f),
                    start=(si == 0),
                    stop=(si == nsi - 1),
                )

        def pv_finish(h, pos):
            """Per f-block: copy O_unnorm^T + denom out of PSUM (ScalarE,
            frees the accumulator bank), run the denominator chain (DMA
            reshape -> DVE reciprocal -> DMA partition-broadcast) and
            normalize on GpSimd.  f-granular so the last head's chain
            pipelines with the output projection."""
            hp, hh = h // 2, (h % 2) * 64
            ou = oupool.tile([65, s], F32, tag="ou", name=f"ou_{h}")
            for f in range(nf):
                nc.scalar.copy(ou[:, ts(f, 512)], pos[f][0:65, :])
                dscr = dpool.tile([1, 512], F32, tag="dscr", name=f"dscr_{h}_{f}")
                nc.sync.dma_start(dscr[:], ou[ds(64, 1), ts(f, 512)])
                d16 = bpool.tile([16, 32], F32, tag="d16", name=f"d16_{h}_{f}")
                nc.sync.dma_start(
                    d16[:], dscr[:].rearrange("one (p c) -> (one p) c", p=16)
                )
                r16 = bpool.tile([16, 32], F32, tag="r16", name=f"r16_{h}_{f}")
                nc.vector.reciprocal(r16[:], d16[:])
                dsc2 = dpool.tile([1, 512], F32, tag="dsc2", name=f"dsc2_{h}_{f}")
                nc.sync.dma_start(
                    dsc2[:].rearrange("one (p c) -> (one p) c", p=16), r16[:]
                )
                bsb = bpool.tile([64, 512], F32, tag="bsb", name=f"bsb_{h}_{f}")
                nc.sync.dma_start(bsb[:], dsc2[:].to_broadcast((64, 512)))
                eng = nc.gpsimd if f % 2 == 0 else nc.vector
                eng.tensor_tensor(
                    O[ds(hh, 64), hp, ts(f, 512)],
                    ou[0:64, ts(f, 512)],
                    bsb[:],
                    ALU.mult,
                )

        # ---- phase A: Q'/K' projections -------------------------------
        for xdram, wdram, dst, bias in ((qT, wq, QT, BQ), (kT, wk, KT, None)):
            wt = wpool.tile([128, 8, 512], BF16, tag="w")
            for ki in range(8):
                nc.sync.dma_start(wt[:, ki, :], wdram[ds(ki * 128, 128), :])
            for fp in range(nfp):
                xts = []
                for ki in range(8):
                    xt = xpool.tile([128, 1024], BF16, tag="x")
                    dma_engines[ki % 2].dma_start(
                        xt[:], xdram[ds(ki * 128, 128), ds(fp * 1024, 1024)]
                    )
                    xts.append(xt)
                for pc in range(4):
                    for half in range(2):
                        ps = pspool.tile([128, 512], F32, tag="ps")
                        for ki in range(8):
                            nc.tensor.matmul(
                                ps[:],
                                lhsT=wt[:, ki, ts(pc, 128)],
                                rhs=xts[ki][:, ts(half, 512)],
                                start=(ki == 0),
                                stop=(ki == 7),
                            )
                        f = 2 * fp + half
                        if bias is not None:
                            nc.vector.tensor_scalar_add(
                                dst[:, pc, ts(f, 512)], ps[:],
                                bias[:, pc : pc + 1],
                            )
                        else:
                            nc.scalar.copy(dst[:, pc, ts(f, 512)], ps[:])

        # ---- w0: V projection + pair-0 QK/exp + head-0 PV -------------
        wtv = wpool.tile([128, 8, 512], BF16, tag="w")
        for ki in range(8):
            nc.sync.dma_start(wtv[:, ki, :], wv[ds(ki * 128, 128), :])
        pos_cur = [
            pspool.tile([128, 512], F32, tag="ps", name=f"pos_0_{i}")
            for i in range(nf)
        ]
        xv = None
        for si in range(nsi):
            fv, sj = si // 8, si % 8
            if sj == 0:
                xv = []
                for ki in range(8):
                    xt = xpool.tile([128, 1024], BF16, tag="x")
                    dma_engines[ki % 2].dma_start(
                        xt[:], vT[ds(ki * 128, 128), ds(fv * 1024, 1024)]
                    )
                    xv.append(xt)
            vps = pss.tile([128, 512], F32, tag="s", name=f"vps_{si}")
            for ki in range(8):
                nc.tensor.matmul(
                    vps[:],
                    lhsT=xv[ki][:, ts(sj, 128)],
                    rhs=wtv[:, ki, :],
                    start=(ki == 0),
                    stop=(ki == 7),
                )
            nc.scalar.copy(
                V[:, si, 0:8, 0:64], vps[:].rearrange("p (h d) -> p h d", h=8)
            )
            emit_qk(0, si)
            if si > 0:
                emit_pv(0, si - 1, pos_cur)
        emit_pv(0, nsi - 1, pos_cur)
        pv_finish(0, pos_cur)

        for ki in range(4):
            nc.sync.dma_start(WO[:, ki, :], wo[ds(ki * 128, 128), :])

        # ---- w1..w7: drain heads 1..7, produce pairs 1..3 at half rate
        qk_chunks = [(p, si) for p in range(1, 4) for si in range(nsi)]
        qi = 0
        for hd in range(1, 8):
            pos_cur = [
                pspool.tile([128, 512], F32, tag="ps", name=f"pos_{hd}_{i}")
                for i in range(nf)
            ]
            for si in range(nsi):
                emit_pv(hd, si, pos_cur)
                if si % 2 == 0 and qi < len(qk_chunks):
                    emit_qk(*qk_chunks[qi])
                    qi += 1
            if hd == 7:
                # bridge the w7 -> out-proj boundary: these matmuls only
                # need heads 0-5, so they run (from the idle pss banks)
                # while head 7's normalize chain completes, keeping the
                # PE clock warm.
                bridge = []
                for pe in range(4):
                    bps = pss.tile([128, 512], F32, tag="s", name=f"br_{pe}")
                    for ki in range(3):
                        nc.tensor.matmul(
                            bps[:],
                            lhsT=WO[:, ki, ts(pe, 128)],
                            rhs=O[:, ki, ts(0, 512)],
                            start=(ki == 0),
                            stop=False,
                        )
                    bridge.append(bps)
            pv_finish(hd, pos_cur)
        ph2b.close()
        ph1.close()

        # ---- phase C: output projection (partial over this core's heads)
        opool = ctx.enter_context(tc.tile_pool(name="ostage", bufs=3))
        outr = outT.rearrange("(o p) n -> p o n", p=128)
        for pe in range(4):
            nc.tensor.matmul(
                bridge[pe],
                lhsT=WO[:, 3, ts(pe, 128)],
                rhs=O[:, 3, ts(0, 512)],
                start=False,
                stop=True,
            )
            ot = opool.tile([128, 512], F32, tag="ot")
            if pe % 2 == 0:
                nc.vector.tensor_copy(ot[:], bridge[pe])
            else:
                nc.scalar.copy(ot[:], bridge[pe])
            nc.sync.dma_start(outr[:, pe, ts(0, 512)], ot[:])
        for f in range(nf):
            for pe in range(8):
                if f == 0 and pe < 4:
                    continue
                ps = pspool.tile([128, 512], F32, tag="ps")
                for ki in range(4):
                    nc.tensor.matmul(
                        ps[:],
                        lhsT=WO[:, ki, ts(pe, 128)],
                        rhs=O[:, ki, ts(f, 512)],
                        start=(ki == 0),
                        stop=(ki == 3),
                    )
                ot = opool.tile([128, 512], F32, tag="ot")
                if (pe + f) % 2 == 0:
                    nc.vector.tensor_copy(ot[:], ps[:])
                else:
                    nc.scalar.copy(ot[:], ps[:])
                nc.sync.dma_start(outr[:, pe, ts(f, 512)], ot[:])

    nc.compile()
    return nc


_NC_CACHE: dict = {}


def get_nc(s: int = S):
    if s not in _NC_CACHE:
        _NC_CACHE[s] = build_nc(s)
    return _NC_CACHE[s]


def _prep_in_maps(q, k, v, Wq, bq, Wk, Wv, Wo):
    """Host-side shard prep: per-core input dicts (cheap numpy reshapes)."""
    f32 = np.float32
    scale = 1.0 / np.sqrt(DK)
    xT = {}
    for b in range(B):
        xT[b] = (
            np.ascontiguousarray(q[b].T).astype(BF16NP),
            np.ascontiguousarray(k[b].T).astype(BF16NP),
            np.ascontiguousarray(v[b].T).astype(BF16NP),
        )
    per_g = {}
    for g in range(2):
        F = slice(512 * g, 512 * g + 512)
        per_g[g] = dict(
            wq=np.ascontiguousarray(Wq[F].T * scale).astype(BF16NP),
            wk=np.ascontiguousarray(Wk[F].T).astype(BF16NP),
            wv=np.ascontiguousarray(Wv[F].T).astype(BF16NP),
            wo=np.ascontiguousarray(Wo[:, F].T).astype(BF16NP),
            bq=np.ascontiguousarray(
                (bq[F] * scale).reshape(4, 128).T, dtype=f32
            ),
        )
    in_maps = []
    for c in range(N_CORES):
        b, g = c // 2, c % 2
        qb, kb, vb = xT[b]
        in_maps.append(dict(qT=qb, kT=kb, vT=vb, **per_g[g]))
    return in_maps


def kernel(q, k, v, Wq, bq, Wk, bk, Wv, bv, Wo, bo):
    q, k, v = (np.asarray(x, np.float32) for x in (q, k, v))
    Wq, bq, Wk, bk = (np.asarray(x, np.float32) for x in (Wq, bq, Wk, bk))
    Wv, bv, Wo, bo = (np.asarray(x, np.float32) for x in (Wv, bv, Wo, bo))

    nc = get_nc(S)
    in_maps = _prep_in_maps(q, k, v, Wq, bq, Wk, Wv, Wo)
    res = run_bass_kernel_spmd(nc, in_maps, core_ids=list(range(N_CORES)))

    # bk drops out of softmax; bv folds into an effective output bias.
    bo_eff = (
        bo.astype(np.float64) + Wo.astype(np.float64) @ bv.astype(np.float64)
    ).astype(np.float32)
    out = np.empty((B, S, D), np.float32)
    for b in range(B):
        acc = res.results[2 * b]["outT"] + res.results[2 * b + 1]["outT"]
        out[b] = acc.T + bo_eff
    return out


# revision 13
# speedup vs baseline: 1.0665x; 1.0149x over previous
"""Multi-head attention kernel for Trainium2 (Bass/Tile), 8 NeuronCores.

Problem: nn_MultiHeadAttention  (B=4, S=2048, D=1024, H=16, DK=64)
    out = softmax((q Wq^T + bq)(k Wk^T + bk)^T / sqrt(DK)) (v Wv^T + bv) Wo^T + bo

Sharding: core c = 2*b + g handles batch b and head-group g (8 heads = 512
features).  Each core computes its batch's attention for its heads plus a
partial output projection; the host sums the two partials per batch.

Math simplifications done on the host (exact):
  - k-bias bk drops out (softmax is shift invariant along the key axis).
  - v-bias bv folds into an effective output bias bo_eff = bo + Wo @ bv.
  - the 1/sqrt(DK) logit scale is folded into Wq/bq.

v3 performance structure (per core):
  - QK^T matmuls run as row-tiled head PAIRS: heads (2p, 2p+1) occupy
    partition halves 0-63 / 64-127 of Q^T/K^T, so their matmuls land on
    disjoint row-halves of the PE array (tile_position (0,0) vs (64,0))
    and execute concurrently -> ~2x on the logit matmuls.
  - exp is split across two engines: ScalarE runs the exact ACT Exp for
    9/16 of tiles, the DVE computes 7/16 with a Schraudolph fast exp
    (int16(x*128/ln2 + (127*128 - C)) bit-cast to bf16).
  - PV, QKV projections and the output projection use N=1024 moving
    operands (half the matmul/LDWEIGHTS count of N=512).
  - pipeline: w0 interleaves the V projection, pair-0 QK+exp, and head-0
    PV; w1..w6 drain one head's PV while producing the next pair's QK at
    half rate; w7 drains head 7; then the output projection.
  - PSUM: 4 banks rotate QK pair tiles (pss, [128,512]), 4 banks hold two
    [*,1024] tiles (pspool): projection psums / PV accumulators.  V
    carries a ones column so the PV matmul emits the softmax denominator
    for free (row 64).
"""

import numpy as np
import ml_dtypes
from contextlib import ExitStack

import concourse.bass as bass
import concourse.tile as tile
from concourse import bacc, mybir
from concourse.bass import ts, ds
from concourse.bass_utils import run_bass_kernel_spmd

B, S, D, H, DK = 4, 2048, 1024, 16, 64
N_CORES = 8
F32 = mybir.dt.float32
BF16 = mybir.dt.bfloat16
I16 = mybir.dt.int16
AF = mybir.ActivationFunctionType
ALU = mybir.AluOpType
BF16NP = ml_dtypes.bfloat16

# Schraudolph fast-exp constants (bf16 bit pattern via int16):
#   E = bitcast_bf16(int16(x * 128/ln2 + (127*128 - C)))
SCH_A = 128.0 / float(np.log(2.0))
SCH_C = 5.8
SCH_B = 127.0 * 128.0 - SCH_C
# fraction of E tiles computed on the DVE (Bresenham NUM/DEN)
DVE_NUM, DVE_DEN = 33, 64


def build_nc(s: int = S):
    """Build + compile the per-core Bass module (SPMD: same NEFF, per-core data)."""
    assert s % 1024 == 0  # x tiles are [128,1024]
    nsi = s // 128   # 128-row key chunks
    nf = s // 512    # 512-col query chunks
    nfp = s // 1024  # 1024-col query pair-chunks

    nc = bacc.Bacc("TRN2", target_bir_lowering=False, debug=False)

    qT = nc.dram_tensor("qT", [D, s], BF16, kind="ExternalInput").ap()
    kT = nc.dram_tensor("kT", [D, s], BF16, kind="ExternalInput").ap()
    vT = nc.dram_tensor("vT", [D, s], BF16, kind="ExternalInput").ap()
    wq = nc.dram_tensor("wq", [D, 512], BF16, kind="ExternalInput").ap()
    wk = nc.dram_tensor("wk", [D, 512], BF16, kind="ExternalInput").ap()
    wv = nc.dram_tensor("wv", [D, 512], BF16, kind="ExternalInput").ap()
    wo = nc.dram_tensor("wo", [512, D], BF16, kind="ExternalInput").ap()
    bq = nc.dram_tensor("bq", [128, 4], F32, kind="ExternalInput").ap()
    outT = nc.dram_tensor("outT", [D, s], F32, kind="ExternalOutput").ap()

    with tile.TileContext(nc) as tc, ExitStack() as ctx:
        pers = ctx.enter_context(tc.tile_pool(name="pers", bufs=1))
        pspool = ctx.enter_context(tc.tile_pool(name="ps", bufs=4, space="PSUM"))
        pss = ctx.enter_context(tc.tile_pool(name="pss", bufs=4, space="PSUM"))
        epool = ctx.enter_context(tc.tile_pool(name="e", bufs=72))

        QT = pers.tile([128, 4, s], BF16)       # Q'^T  [feature, seq]
        KT = pers.tile([128, 4, s], BF16)       # K^T   [feature, seq]
        # V nat [seq, head, dv|ones|pad] + ghost 9th head so PV lhsT can be
        # padded to 128 columns (full-width weights enable fast weight load)
        Vt = pers.tile([128, nsi, 9 * 66], BF16)
        V = Vt.rearrange("p n (h e) -> p n h e", h=9)
        O = pers.tile([128, 4, s], BF16)        # O^T normalized
        WO = pers.tile([128, 4, D], BF16)
        BQ = pers.tile([128, 4], F32)

        nc.sync.dma_start(BQ[:], bq)
        nc.vector.memset(V[:, :, 0:8, 64:66], 1.0)
        nc.vector.memset(V[:, :, 8, :], 0.0)

        # PE warm-up: dependency-free matmuls on a zeroed scratch tile ramp
        # the HAM clock gate while the first input DMAs are still in flight.
        scratch = epool.tile([128, 512], BF16, tag="e", name="warmup_scratch")
        nc.vector.memset(scratch[:], 0.0)
        wps = pss.tile([128, 512], F32, tag="s", name="warmup_ps")
        for _ in range(24):
            nc.tensor.matmul(
                wps[:], lhsT=scratch[:, 0:128], rhs=scratch[:],
                start=True, stop=True, skip_group_check=True,
            )

        ph1 = ExitStack()
        xpool = ph1.enter_context(tc.tile_pool(name="x", bufs=8))
        wpool = ph1.enter_context(tc.tile_pool(name="w", bufs=2))
        ph2b = ExitStack()
        bpool = ph2b.enter_context(tc.tile_pool(name="b", bufs=2))
        oupool = ph2b.enter_context(tc.tile_pool(name="ou", bufs=1))
        dpool = ph2b.enter_context(tc.tile_pool(name="dscr", bufs=4, space="DRAM"))

        # E[h][si][f] -> AP (bf16 view) for the PV matmuls
        E: dict = {h: {} for h in range(8)}
        expc = [0]  # exp tile counter for the engine split

        def emit_exp(h, si, f, ps):
            c = expc[0]
            expc[0] += 1
            use_dve = ((c + 1) * DVE_NUM) // DVE_DEN > (c * DVE_NUM) // DVE_DEN
            if use_dve:
                e = epool.tile([128, 512], I16, tag="e", name=f"e_{h}_{si}_{f}")
                nc.vector.tensor_scalar(
                    e[:], ps[:], SCH_A, SCH_B, ALU.mult, ALU.add
                )
                E[h].setdefault(si, {})[f] = e.bitcast(BF16)
            else:
                e = epool.tile([128, 512], BF16, tag="e", name=f"e_{h}_{si}_{f}")
                nc.scalar.activation(e[:], ps[:], AF.Exp)
                E[h].setdefault(si, {})[f] = e[:]

        def emit_qk(p, si):
            """Row-tiled pair: head 2p on PE rows 0-63, head 2p+1 on 64-127."""
            for f in range(nf):
                psA = pss.tile([128, 512], F32, tag="s", name=f"sA_{p}_{si}_{f}")
                psB = pss.tile([128, 512], F32, tag="s", name=f"sB_{p}_{si}_{f}")
                nc.tensor.matmul(
                    psA[:], lhsT=KT[ds(0, 64), p, ts(si, 128)],
                    rhs=QT[ds(0, 64), p, ts(f, 512)], start=True, stop=True,
                )
                nc.tensor.matmul(
                    psB[:], lhsT=KT[ds(64, 64), p, ts(si, 128)],
                    rhs=QT[ds(64, 64), p, ts(f, 512)], start=True, stop=True,
                )
                emit_exp(2 * p, si, f, psA)
                emit_exp(2 * p + 1, si, f, psB)

        def emit_pv(h, si, pos):
            for f in range(nf):
                nc.tensor.matmul(
                    pos[f][:, :],
                    lhsT=Vt[:, si, ds(h * 66, 128)],
                    rhs=E[h][si].pop(f),
                    start=(si == 0),
                    stop=(si == nsi - 1),
                )

        def pv_finish(h, pos):
            """Per f-block: copy O_unnorm^T + denom out of PSUM (ScalarE,
            frees the accumulator bank), run the denominator chain (DMA
            reshape -> DVE reciprocal -> DMA partition-broadcast) and
            normalize on GpSimd.  f-granular so the last head's chain
            pipelines with the output projection."""
            hp, hh = h // 2, (h % 2) * 64
            ou = oupool.tile([65, s], F32, tag="ou", name=f"ou_{h}")
            for f in range(nf):
                nc.scalar.copy(ou[:, ts(f, 512)], pos[f][0:65, :])
                dscr = dpool.tile([1, 512], F32, tag="dscr", name=f"dscr_{h}_{f}")
                nc.sync.dma_start(dscr[:], ou[ds(64, 1), ts(f, 512)])
                d16 = bpool.tile([16, 32], F32, tag="d16", name=f"d16_{h}_{f}")
                nc.sync.dma_start(
                    d16[:], dscr[:].rearrange("one (p c) -> (one p) c", p=16)
                )
                r16 = bpool.tile([16, 32], F32, tag="r16", name=f"r16_{h}_{f}")
                nc.vector.reciprocal(r16[:], d16[:])
                dsc2 = dpool.tile([1, 512], F32, tag="dsc2", name=f"dsc2_{h}_{f}")
                nc.sync.dma_start(
                    dsc2[:].rearrange("one (p c) -> (one p) c", p=16), r16[:]
                )
                bsb = bpool.tile([64, 512], F32, tag="bsb", name=f"bsb_{h}_{f}")
                nc.sync.dma_start(bsb[:], dsc2[:].to_broadcast((64, 512)))
                nc.gpsimd.tensor_tensor(
                    O[ds(hh, 64), hp, ts(f, 512)],
                    ou[0:64, ts(f, 512)],
                    bsb[:],
                    ALU.mult,
                )

        # ---- phase A: Q'/K' projections -------------------------------
        for xdram, wdram, dst, bias in ((qT, wq, QT, BQ), (kT, wk, KT, None)):
            wt = wpool.tile([128, 8, 512], BF16, tag="w")
            for ki in range(8):
                nc.sync.dma_start(wt[:, ki, :], wdram[ds(ki * 128, 128), :])
            for fp in range(nfp):
                xts = []
                for ki in range(8):
                    xt = xpool.tile([128, 1024], BF16, tag="x")
                    eng = nc.sync if ki % 2 == 0 else nc.scalar
                    eng.dma_start(
                        xt[:], xdram[ds(ki * 128, 128), ds(fp * 1024, 1024)]
                    )
                    xts.append(xt)
                for pc in range(4):
                    for half in range(2):
                        ps = pspool.tile([128, 512], F32, tag="ps")
                        for ki in range(8):
                            nc.tensor.matmul(
                                ps[:],
                                lhsT=wt[:, ki, ts(pc, 128)],
                                rhs=xts[ki][:, ts(half, 512)],
                                start=(ki == 0),
                                stop=(ki == 7),
                            )
                        f = 2 * fp + half
                        if bias is not None:
                            nc.vector.tensor_scalar_add(
                                dst[:, pc, ts(f, 512)], ps[:],
                                bias[:, pc : pc + 1],
                            )
                        else:
                            nc.scalar.copy(dst[:, pc, ts(f, 512)], ps[:])

        # ---- w0: V projection + pair-0 QK/exp + head-0 PV -------------
        wtv = wpool.tile([128, 8, 512], BF16, tag="w")
        for ki in range(8):
            nc.sync.dma_start(wtv[:, ki, :], wv[ds(ki * 128, 128), :])
        pos_cur = [
            pspool.tile([128, 512], F32, tag="ps", name=f"pos_0_{i}")
            for i in range(nf)
        ]
        xv = None
        for si in range(nsi):
            fv, sj = si // 8, si % 8
            if sj == 0:
                xv = []
                for ki in range(8):
                    xt = xpool.tile([128, 1024], BF16, tag="x")
                    nc.sync.dma_start(
                        xt[:], vT[ds(ki * 128, 128), ds(fv * 1024, 1024)]
                    )
                    xv.append(xt)
            vps = pss.tile([128, 512], F32, tag="s", name=f"vps_{si}")
            for ki in range(8):
                nc.tensor.matmul(
                    vps[:],
                    lhsT=xv[ki][:, ts(sj, 128)],
                    rhs=wtv[:, ki, :],
                    start=(ki == 0),
                    stop=(ki == 7),
                )
            nc.scalar.copy(
                V[:, si, 0:8, 0:64], vps[:].rearrange("p (h d) -> p h d", h=8)
            )
            emit_qk(0, si)
            if si > 0:
                emit_pv(0, si - 1, pos_cur)
        emit_pv(0, nsi - 1, pos_cur)
        pv_finish(0, pos_cur)

        for ki in range(4):
            nc.sync.dma_start(WO[:, ki, :], wo[ds(ki * 128, 128), :])

        # ---- w1..w7: drain heads 1..7, produce pairs 1..3 at half rate
        qk_chunks = [(p, si) for p in range(1, 4) for si in range(nsi)]
        qi = 0
        for hd in range(1, 8):
            pos_cur = [
                pspool.tile([128, 512], F32, tag="ps", name=f"pos_{hd}_{i}")
                for i in range(nf)
            ]
            for si in range(nsi):
                emit_pv(hd, si, pos_cur)
                if si % 2 == 0 and qi < len(qk_chunks):
                    emit_qk(*qk_chunks[qi])
                    qi += 1
            if hd == 7:
                # bridge the w7 -> out-proj boundary: these matmuls only
                # need heads 0-5, so they run (from the idle pss banks)
                # while head 7's normalize chain completes, keeping the
                # PE clock warm.
                bridge = []
                for pe in range(4):
                    bps = pss.tile([128, 512], F32, tag="s", name=f"br_{pe}")
                    for ki in range(3):
                        nc.tensor.matmul(
                            bps[:],
                            lhsT=WO[:, ki, ts(pe, 128)],
                            rhs=O[:, ki, ts(0, 512)],
                            start=(ki == 0),
                            stop=False,
                        )
                    bridge.append(bps)
            pv_finish(hd, pos_cur)
        ph2b.close()
        ph1.close()

        # ---- phase C: output projection (partial over this core's heads)
        opool = ctx.enter_context(tc.tile_pool(name="ostage", bufs=3))
        outr = outT.rearrange("(o p) n -> p o n", p=128)
        for pe in range(4):
            nc.tensor.matmul(
                bridge[pe],
                lhsT=WO[:, 3, ts(pe, 128)],
                rhs=O[:, 3, ts(0, 512)],
                start=False,
                stop=True,
            )
            ot = opool.tile([128, 512], F32, tag="ot")
            nc.vector.tensor_copy(ot[:], bridge[pe])
            (nc.sync if pe % 2 == 0 else nc.scalar).dma_start(
                outr[:, pe, ts(0, 512)], ot[:]
            )
        for f in range(nf):
            for pe in range(8):
                if f == 0 and pe < 4:
                    continue
                ps = pspool.tile([128, 512], F32, tag="ps")
                for ki in range(4):
                    nc.tensor.matmul(
                        ps[:],
                        lhsT=WO[:, ki, ts(pe, 128)],
                        rhs=O[:, ki, ts(f, 512)],
                        start=(ki == 0),
                        stop=(ki == 3),
                    )
                ot = opool.tile([128, 512], F32, tag="ot")
                if (pe + f) % 2 == 0:
                    nc.vector.tensor_copy(ot[:], ps[:])
                else:
                    nc.scalar.copy(ot[:], ps[:])
                (nc.sync if (pe + f) % 2 == 0 else nc.scalar).dma_start(
                    outr[:, pe, ts(f, 512)], ot[:]
                )

    nc.compile()
    return nc


_NC_CACHE: dict = {}


def get_nc(s: int = S):
    if s not in _NC_CACHE:
        _NC_CACHE[s] = build_nc(s)
    return _NC_CACHE[s]


def _prep_in_maps(q, k, v, Wq, bq, Wk, Wv, Wo):
    """Host-side shard prep: per-core input dicts (cheap numpy reshapes)."""
    f32 = np.float32
    scale = 1.0 / np.sqrt(DK)
    xT = {}
    for b in range(B):
        xT[b] = (
            np.ascontiguousarray(q[b].T).astype(BF16NP),
            np.ascontiguousarray(k[b].T).astype(BF16NP),
            np.ascontiguousarray(v[b].T).astype(BF16NP),
        )
    per_g = {}
    for g in range(2):
        F = slice(512 * g, 512 * g + 512)
        per_g[g] = dict(
            wq=np.ascontiguousarray(Wq[F].T * scale).astype(BF16NP),
            wk=np.ascontiguousarray(Wk[F].T).astype(BF16NP),
            wv=np.ascontiguousarray(Wv[F].T).astype(BF16NP),
            wo=np.ascontiguousarray(Wo[:, F].T).astype(BF16NP),
            bq=np.ascontiguousarray(
                (bq[F] * scale).reshape(4, 128).T, dtype=f32
            ),
        )
    in_maps = []
    for c in range(N_CORES):
        b, g = c // 2, c % 2
        qb, kb, vb = xT[b]
        in_maps.append(dict(qT=qb, kT=kb, vT=vb, **per_g[g]))
    return in_maps


def kernel(q, k, v, Wq, bq, Wk, bk, Wv, bv, Wo, bo):
    q, k, v = (np.asarray(x, np.float32) for x in (q, k, v))
    Wq, bq, Wk, bk = (np.asarray(x, np.float32) for x in (Wq, bq, Wk, bk))
    Wv, bv, Wo, bo = (np.asarray(x, np.float32) for x in (Wv, bv, Wo, bo))

    nc = get_nc(S)
    in_maps = _prep_in_maps(q, k, v, Wq, bq, Wk, Wv, Wo)
    res = run_bass_kernel_spmd(nc, in_maps, core_ids=list(range(N_CORES)))

    # bk drops out of softmax; bv folds into an effective output bias.
    bo_eff = (
        bo.astype(np.float64) + Wo.astype(np.float64) @ bv.astype(np.float64)
    ).astype(np.float32)
    out = np.empty((B, S, D), np.float32)
    for b in range(B):
        acc = res.results[2 * b]["outT"] + res.results[2 * b + 1]["outT"]
        out[b] = acc.T + bo_eff
    return out


# revision 17
# speedup vs baseline: 1.0950x; 1.0267x over previous
"""Multi-head attention kernel for Trainium2 (Bass/Tile), 8 NeuronCores.

Problem: nn_MultiHeadAttention  (B=4, S=2048, D=1024, H=16, DK=64)
    out = softmax((q Wq^T + bq)(k Wk^T + bk)^T / sqrt(DK)) (v Wv^T + bv) Wo^T + bo

Sharding: core c = 2*b + g handles batch b and head-group g (8 heads = 512
features).  Each core computes its batch's attention for its heads plus a
partial output projection; the host sums the two partials per batch.

Math simplifications done on the host (exact):
  - k-bias bk drops out (softmax is shift invariant along the key axis).
  - v-bias bv folds into an effective output bias bo_eff = bo + Wo @ bv.
  - the 1/sqrt(DK) logit scale is folded into Wq/bq.

v3 performance structure (per core):
  - QK^T matmuls run as row-tiled head PAIRS: heads (2p, 2p+1) occupy
    partition halves 0-63 / 64-127 of Q^T/K^T, so their matmuls land on
    disjoint row-halves of the PE array (tile_position (0,0) vs (64,0))
    and execute concurrently -> ~2x on the logit matmuls.
  - exp is split across two engines: ScalarE runs the exact ACT Exp for
    9/16 of tiles, the DVE computes 7/16 with a Schraudolph fast exp
    (int16(x*128/ln2 + (127*128 - C)) bit-cast to bf16).
  - PV, QKV projections and the output projection use N=1024 moving
    operands (half the matmul/LDWEIGHTS count of N=512).
  - pipeline: w0 interleaves the V projection, pair-0 QK+exp, and head-0
    PV; w1..w6 drain one head's PV while producing the next pair's QK at
    half rate; w7 drains head 7; then the output projection.
  - PSUM: 4 banks rotate QK pair tiles (pss, [128,512]), 4 banks hold two
    [*,1024] tiles (pspool): projection psums / PV accumulators.  V
    carries a ones column so the PV matmul emits the softmax denominator
    for free (row 64).
"""

import numpy as np
import ml_dtypes
from contextlib import ExitStack

import concourse.bass as bass
import concourse.tile as tile
from concourse import bacc, mybir
from concourse.bass import ts, ds
from concourse.bass_utils import run_bass_kernel_spmd

B, S, D, H, DK = 4, 2048, 1024, 16, 64
N_CORES = 8
F32 = mybir.dt.float32
BF16 = mybir.dt.bfloat16
I16 = mybir.dt.int16
AF = mybir.ActivationFunctionType
ALU = mybir.AluOpType
BF16NP = ml_dtypes.bfloat16

# Schraudolph fast-exp constants (bf16 bit pattern via int16):
#   E = bitcast_bf16(int16(x * 128/ln2 + (127*128 - C)))
SCH_A = 128.0 / float(np.log(2.0))
SCH_C = 5.8
SCH_B = 127.0 * 128.0 - SCH_C
# fraction of E tiles computed on the DVE (Bresenham NUM/DEN)
DVE_NUM, DVE_DEN = 33, 64


def build_nc(s: int = S):
    """Build + compile the per-core Bass module (SPMD: same NEFF, per-core data)."""
    assert s % 1024 == 0  # x tiles are [128,1024]
    nsi = s // 128   # 128-row key chunks
    nf = s // 512    # 512-col query chunks
    nfp = s // 1024  # 1024-col query pair-chunks

    nc = bacc.Bacc("TRN2", target_bir_lowering=False, debug=False)

    qT = nc.dram_tensor("qT", [D, s], BF16, kind="ExternalInput").ap()
    kT = nc.dram_tensor("kT", [D, s], BF16, kind="ExternalInput").ap()
    vT = nc.dram_tensor("vT", [D, s], BF16, kind="ExternalInput").ap()
    wq = nc.dram_tensor("wq", [D, 512], BF16, kind="ExternalInput").ap()
    wk = nc.dram_tensor("wk", [D, 512], BF16, kind="ExternalInput").ap()
    wv = nc.dram_tensor("wv", [D, 512], BF16, kind="ExternalInput").ap()
    wo = nc.dram_tensor("wo", [512, D], BF16, kind="ExternalInput").ap()
    bq = nc.dram_tensor("bq", [128, 4], F32, kind="ExternalInput").ap()
    outT = nc.dram_tensor("outT", [D, s], F32, kind="ExternalOutput").ap()

    with tile.TileContext(nc) as tc, ExitStack() as ctx:
        pers = ctx.enter_context(tc.tile_pool(name="pers", bufs=1))
        pspool = ctx.enter_context(tc.tile_pool(name="ps", bufs=4, space="PSUM"))
        pss = ctx.enter_context(tc.tile_pool(name="pss", bufs=4, space="PSUM"))
        epool = ctx.enter_context(tc.tile_pool(name="e", bufs=72))

        QT = pers.tile([128, 4, s], BF16)       # Q'^T  [feature, seq]
        KT = pers.tile([128, 4, s], BF16)       # K^T   [feature, seq]
        # V nat [seq, head, dv|ones|pad] + ghost 9th head so PV lhsT can be
        # padded to 128 columns (full-width weights enable fast weight load)
        Vt = pers.tile([128, nsi, 9 * 66], BF16)
        V = Vt.rearrange("p n (h e) -> p n h e", h=9)
        O = pers.tile([128, 4, s], BF16)        # O^T normalized
        WO = pers.tile([128, 4, D], BF16)
        BQ = pers.tile([128, 4], F32)

        nc.sync.dma_start(BQ[:], bq)
        nc.vector.memset(V[:, :, 0:8, 64:66], 1.0)
        nc.vector.memset(V[:, :, 8, :], 0.0)

        ph1 = ExitStack()
        xpool = ph1.enter_context(tc.tile_pool(name="x", bufs=8))
        wpool = ph1.enter_context(tc.tile_pool(name="w", bufs=2))
        ph2b = ExitStack()
        bpool = ph2b.enter_context(tc.tile_pool(name="b", bufs=2))
        oupool = ph2b.enter_context(tc.tile_pool(name="ou", bufs=1))
        dpool = ph2b.enter_context(tc.tile_pool(name="dscr", bufs=4, space="DRAM"))

        # E[h][si][f] -> AP (bf16 view) for the PV matmuls
        E: dict = {h: {} for h in range(8)}
        expc = [0]  # exp tile counter for the engine split

        def emit_exp(h, si, f, ps):
            c = expc[0]
            expc[0] += 1
            use_dve = ((c + 1) * DVE_NUM) // DVE_DEN > (c * DVE_NUM) // DVE_DEN
            if use_dve:
                e = epool.tile([128, 512], I16, tag="e", name=f"e_{h}_{si}_{f}")
                nc.vector.tensor_scalar(
                    e[:], ps[:], SCH_A, SCH_B, ALU.mult, ALU.add
                )
                E[h].setdefault(si, {})[f] = e.bitcast(BF16)
            else:
                e = epool.tile([128, 512], BF16, tag="e", name=f"e_{h}_{si}_{f}")
                nc.scalar.activation(e[:], ps[:], AF.Exp)
                E[h].setdefault(si, {})[f] = e[:]

        def emit_qk(p, si, halves=(0, 1)):
            """Row-tiled pair: head 2p on PE rows 0-63, head 2p+1 on 64-127."""
            for hf in halves:
                for f in range(hf * nf // 2, (hf + 1) * nf // 2):
                    psA = pss.tile([128, 512], F32, tag="s", name=f"sA_{p}_{si}_{f}")
                    psB = pss.tile([128, 512], F32, tag="s", name=f"sB_{p}_{si}_{f}")
                    nc.tensor.matmul(
                        psA[:], lhsT=KT[ds(0, 64), p, ts(si, 128)],
                        rhs=QT[ds(0, 64), p, ts(f, 512)], start=True, stop=True,
                    )
                    nc.tensor.matmul(
                        psB[:], lhsT=KT[ds(64, 64), p, ts(si, 128)],
                        rhs=QT[ds(64, 64), p, ts(f, 512)], start=True, stop=True,
                    )
                    emit_exp(2 * p, si, f, psA)
                    emit_exp(2 * p + 1, si, f, psB)

        def emit_pv(h, si, pos):
            for f in range(nf):
                nc.tensor.matmul(
                    pos[f][:, :],
                    lhsT=Vt[:, si, ds(h * 66, 128)],
                    rhs=E[h][si].pop(f),
                    start=(si == 0),
                    stop=(si == nsi - 1),
                )

        def pv_finish(h, pos):
            """Per f-block: copy O_unnorm^T + denom out of PSUM (ScalarE,
            frees the accumulator bank), run the denominator chain (DMA
            reshape -> DVE reciprocal -> DMA partition-broadcast) and
            normalize on GpSimd.  f-granular so the last head's chain
            pipelines with the output projection."""
            hp, hh = h // 2, (h % 2) * 64
            ou = oupool.tile([65, s], F32, tag="ou", name=f"ou_{h}")
            for f in range(nf):
                nc.scalar.copy(ou[:, ts(f, 512)], pos[f][0:65, :])
                dscr = dpool.tile([1, 512], F32, tag="dscr", name=f"dscr_{h}_{f}")
                nc.sync.dma_start(dscr[:], ou[ds(64, 1), ts(f, 512)])
                d16 = bpool.tile([16, 32], F32, tag="d16", name=f"d16_{h}_{f}")
                nc.sync.dma_start(
                    d16[:], dscr[:].rearrange("one (p c) -> (one p) c", p=16)
                )
                r16 = bpool.tile([16, 32], F32, tag="r16", name=f"r16_{h}_{f}")
                nc.vector.reciprocal(r16[:], d16[:])
                dsc2 = dpool.tile([1, 512], F32, tag="dsc2", name=f"dsc2_{h}_{f}")
                nc.sync.dma_start(
                    dsc2[:].rearrange("one (p c) -> (one p) c", p=16), r16[:]
                )
                bsb = bpool.tile([64, 512], F32, tag="bsb", name=f"bsb_{h}_{f}")
                nc.sync.dma_start(bsb[:], dsc2[:].to_broadcast((64, 512)))
                nc.gpsimd.tensor_tensor(
                    O[ds(hh, 64), hp, ts(f, 512)],
                    ou[0:64, ts(f, 512)],
                    bsb[:],
                    ALU.mult,
                )

        # ---- phase A: Q'/K' projections -------------------------------
        for xdram, wdram, dst, bias in ((qT, wq, QT, BQ), (kT, wk, KT, None)):
            wt = wpool.tile([128, 8, 512], BF16, tag="w")
            for ki in range(8):
                nc.sync.dma_start(wt[:, ki, :], wdram[ds(ki * 128, 128), :])
            for fp in range(nfp):
                xts = []
                for ki in range(8):
                    xt = xpool.tile([128, 1024], BF16, tag="x")
                    nc.sync.dma_start(
                        xt[:], xdram[ds(ki * 128, 128), ds(fp * 1024, 1024)]
                    )
                    xts.append(xt)
                for pc in range(4):
                    for half in range(2):
                        ps = pspool.tile([128, 512], F32, tag="ps")
                        for ki in range(8):
                            nc.tensor.matmul(
                                ps[:],
                                lhsT=wt[:, ki, ts(pc, 128)],
                                rhs=xts[ki][:, ts(half, 512)],
                                start=(ki == 0),
                                stop=(ki == 7),
                            )
                        f = 2 * fp + half
                        if bias is not None:
                            nc.vector.tensor_scalar_add(
                                dst[:, pc, ts(f, 512)], ps[:],
                                bias[:, pc : pc + 1],
                            )
                        else:
                            nc.scalar.copy(dst[:, pc, ts(f, 512)], ps[:])

        # ---- w0: V projection + pair-0 QK/exp + head-0 PV -------------
        wtv = wpool.tile([128, 8, 512], BF16, tag="w")
        for ki in range(8):
            nc.sync.dma_start(wtv[:, ki, :], wv[ds(ki * 128, 128), :])
        pos_cur = [
            pspool.tile([128, 512], F32, tag="ps", name=f"pos_0_{i}")
            for i in range(nf)
        ]
        xv = None
        for si in range(nsi):
            fv, sj = si // 8, si % 8
            if sj == 0:
                xv = []
                for ki in range(8):
                    xt = xpool.tile([128, 1024], BF16, tag="x")
                    nc.sync.dma_start(
                        xt[:], vT[ds(ki * 128, 128), ds(fv * 1024, 1024)]
                    )
                    xv.append(xt)
            vps = pss.tile([128, 512], F32, tag="s", name=f"vps_{si}")
            for ki in range(8):
                nc.tensor.matmul(
                    vps[:],
                    lhsT=xv[ki][:, ts(sj, 128)],
                    rhs=wtv[:, ki, :],
                    start=(ki == 0),
                    stop=(ki == 7),
                )
            nc.scalar.copy(
                V[:, si, 0:8, 0:64], vps[:].rearrange("p (h d) -> p h d", h=8)
            )
            emit_qk(0, si, halves=(0,))
            if si > 0:
                emit_pv(0, si - 1, pos_cur)
            emit_qk(0, si, halves=(1,))
        emit_pv(0, nsi - 1, pos_cur)
        pv_finish(0, pos_cur)

        for ki in range(4):
            nc.sync.dma_start(WO[:, ki, :], wo[ds(ki * 128, 128), :])

        # ---- w1..w7: drain heads 1..7, produce pairs 1..3 at half rate
        qk_chunks = [(p, si) for p in range(1, 4) for si in range(nsi)]
        qi = 0
        for hd in range(1, 8):
            pos_cur = [
                pspool.tile([128, 512], F32, tag="ps", name=f"pos_{hd}_{i}")
                for i in range(nf)
            ]
            for si in range(nsi):
                chunk = qk_chunks[qi] if (si % 2 == 0 and qi < len(qk_chunks)) else None
                if chunk is not None:
                    emit_qk(*chunk, halves=(0,))
                    qi += 1
                emit_pv(hd, si, pos_cur)
                if chunk is not None:
                    emit_qk(*chunk, halves=(1,))
            pv_finish(hd, pos_cur)
        ph2b.close()
        ph1.close()

        # ---- phase C: output projection (partial over this core's heads)
        opool = ctx.enter_context(tc.tile_pool(name="ostage", bufs=3))
        outr = outT.rearrange("(o p) n -> p o n", p=128)
        for f in range(nf):
            for pe in range(8):
                ps = pspool.tile([128, 512], F32, tag="ps")
                for ki in range(4):
                    nc.tensor.matmul(
                        ps[:],
                        lhsT=WO[:, ki, ts(pe, 128)],
                        rhs=O[:, ki, ts(f, 512)],
                        start=(ki == 0),
                        stop=(ki == 3),
                    )
                ot = opool.tile([128, 512], F32, tag="ot")
                if (pe + f) % 2 == 0:
                    nc.vector.tensor_copy(ot[:], ps[:])
                else:
                    nc.scalar.copy(ot[:], ps[:])
                nc.sync.dma_start(outr[:, pe, ts(f, 512)], ot[:])

    nc.compile()
    return nc


_NC_CACHE: dict = {}


def get_nc(s: int = S):
    if s not in _NC_CACHE:
        _NC_CACHE[s] = build_nc(s)
    return _NC_CACHE[s]


def _prep_in_maps(q, k, v, Wq, bq, Wk, Wv, Wo):
    """Host-side shard prep: per-core input dicts (cheap numpy reshapes)."""
    f32 = np.float32
    scale = 1.0 / np.sqrt(DK)
    xT = {}
    for b in range(B):
        xT[b] = (
            np.ascontiguousarray(q[b].T).astype(BF16NP),
            np.ascontiguousarray(k[b].T).astype(BF16NP),
            np.ascontiguousarray(v[b].T).astype(BF16NP),
        )
    per_g = {}
    for g in range(2):
        F = slice(512 * g, 512 * g + 512)
        per_g[g] = dict(
            wq=np.ascontiguousarray(Wq[F].T * scale).astype(BF16NP),
            wk=np.ascontiguousarray(Wk[F].T).astype(BF16NP),
            wv=np.ascontiguousarray(Wv[F].T).astype(BF16NP),
            wo=np.ascontiguousarray(Wo[:, F].T).astype(BF16NP),
            bq=np.ascontiguousarray(
                (bq[F] * scale).reshape(4, 128).T, dtype=f32
            ),
        )
    in_maps = []
    for c in range(N_CORES):
        b, g = c // 2, c % 2
        qb, kb, vb = xT[b]
        in_maps.append(dict(qT=qb, kT=kb, vT=vb, **per_g[g]))
    return in_maps


def kernel(q, k, v, Wq, bq, Wk, bk, Wv, bv, Wo, bo):
    q, k, v = (np.asarray(x, np.float32) for x in (q, k, v))
    Wq, bq, Wk, bk = (np.asarray(x, np.float32) for x in (Wq, bq, Wk, bk))
    Wv, bv, Wo, bo = (np.asarray(x, np.float32) for x in (Wv, bv, Wo, bo))

    nc = get_nc(S)
    in_maps = _prep_in_maps(q, k, v, Wq, bq, Wk, Wv, Wo)
    res = run_bass_kernel_spmd(nc, in_maps, core_ids=list(range(N_CORES)))

    # bk drops out of softmax; bv folds into an effective output bias.
    bo_eff = (
        bo.astype(np.float64) + Wo.astype(np.float64) @ bv.astype(np.float64)
    ).astype(np.float32)
    out = np.empty((B, S, D), np.float32)
    for b in range(B):
        acc = res.results[2 * b]["outT"] + res.results[2 * b + 1]["outT"]
        out[b] = acc.T + bo_eff
    return out


# revision 19
# speedup vs baseline: 1.0973x; 1.0022x over previous
"""Multi-head attention kernel for Trainium2 (Bass/Tile), 8 NeuronCores.

Problem: nn_MultiHeadAttention  (B=4, S=2048, D=1024, H=16, DK=64)
    out = softmax((q Wq^T + bq)(k Wk^T + bk)^T / sqrt(DK)) (v Wv^T + bv) Wo^T + bo

Sharding: core c = 2*b + g handles batch b and head-group g (8 heads = 512
features).  Each core computes its batch's attention for its heads plus a
partial output projection; the host sums the two partials per batch.

Math simplifications done on the host (exact):
  - k-bias bk drops out (softmax is shift invariant along the key axis).
  - v-bias bv folds into an effective output bias bo_eff = bo + Wo @ bv.
  - the 1/sqrt(DK) logit scale is folded into Wq/bq.

v3 performance structure (per core):
  - QK^T matmuls run as row-tiled head PAIRS: heads (2p, 2p+1) occupy
    partition halves 0-63 / 64-127 of Q^T/K^T, so their matmuls land on
    disjoint row-halves of the PE array (tile_position (0,0) vs (64,0))
    and execute concurrently -> ~2x on the logit matmuls.
  - exp is split across two engines: ScalarE runs the exact ACT Exp for
    9/16 of tiles, the DVE computes 7/16 with a Schraudolph fast exp
    (int16(x*128/ln2 + (127*128 - C)) bit-cast to bf16).
  - PV, QKV projections and the output projection use N=1024 moving
    operands (half the matmul/LDWEIGHTS count of N=512).
  - pipeline: w0 interleaves the V projection, pair-0 QK+exp, and head-0
    PV; w1..w6 drain one head's PV while producing the next pair's QK at
    half rate; w7 drains head 7; then the output projection.
  - PSUM: 4 banks rotate QK pair tiles (pss, [128,512]), 4 banks hold two
    [*,1024] tiles (pspool): projection psums / PV accumulators.  V
    carries a ones column so the PV matmul emits the softmax denominator
    for free (row 64).
"""

import numpy as np
import ml_dtypes
from contextlib import ExitStack

import concourse.bass as bass
import concourse.tile as tile
from concourse import bacc, mybir
from concourse.bass import ts, ds
from concourse.bass_utils import run_bass_kernel_spmd

B, S, D, H, DK = 4, 2048, 1024, 16, 64
N_CORES = 8
F32 = mybir.dt.float32
BF16 = mybir.dt.bfloat16
I16 = mybir.dt.int16
AF = mybir.ActivationFunctionType
ALU = mybir.AluOpType
BF16NP = ml_dtypes.bfloat16

# Schraudolph fast-exp constants (bf16 bit pattern via int16):
#   E = bitcast_bf16(int16(x * 128/ln2 + (127*128 - C)))
SCH_A = 128.0 / float(np.log(2.0))
SCH_C = 5.8
SCH_B = 127.0 * 128.0 - SCH_C
# fraction of E tiles computed on the DVE (Bresenham NUM/DEN)
DVE_NUM, DVE_DEN = 33, 64


def build_nc(s: int = S):
    """Build + compile the per-core Bass module (SPMD: same NEFF, per-core data)."""
    assert s % 1024 == 0  # x tiles are [128,1024]
    nsi = s // 128   # 128-row key chunks
    nf = s // 512    # 512-col query chunks
    nfp = s // 1024  # 1024-col query pair-chunks

    nc = bacc.Bacc("TRN2", target_bir_lowering=False, debug=False)

    qT = nc.dram_tensor("qT", [D, s], BF16, kind="ExternalInput").ap()
    kT = nc.dram_tensor("kT", [D, s], BF16, kind="ExternalInput").ap()
    vT = nc.dram_tensor("vT", [D, s], BF16, kind="ExternalInput").ap()
    wq = nc.dram_tensor("wq", [D, 512], BF16, kind="ExternalInput").ap()
    wk = nc.dram_tensor("wk", [D, 512], BF16, kind="ExternalInput").ap()
    wv = nc.dram_tensor("wv", [D, 512], BF16, kind="ExternalInput").ap()
    wo = nc.dram_tensor("wo", [512, D], BF16, kind="ExternalInput").ap()
    bq = nc.dram_tensor("bq", [128, 4], F32, kind="ExternalInput").ap()
    outT = nc.dram_tensor("outT", [D, s], F32, kind="ExternalOutput").ap()

    with tile.TileContext(nc) as tc, ExitStack() as ctx:
        pers = ctx.enter_context(tc.tile_pool(name="pers", bufs=1))
        pspool = ctx.enter_context(tc.tile_pool(name="ps", bufs=4, space="PSUM"))
        pss = ctx.enter_context(tc.tile_pool(name="pss", bufs=4, space="PSUM"))
        epool = ctx.enter_context(tc.tile_pool(name="e", bufs=72))

        QT = pers.tile([128, 4, s], BF16)       # Q'^T  [feature, seq]
        KT = pers.tile([128, 4, s], BF16)       # K^T   [feature, seq]
        # V nat [seq, head, dv|ones|pad] + ghost 9th head so PV lhsT can be
        # padded to 128 columns (full-width weights enable fast weight load)
        Vt = pers.tile([128, nsi, 9 * 66], BF16)
        V = Vt.rearrange("p n (h e) -> p n h e", h=9)
        O = pers.tile([128, 4, s], BF16)        # O^T normalized
        WO = pers.tile([128, 4, D], BF16)
        BQ = pers.tile([128, 4], F32)

        nc.sync.dma_start(BQ[:], bq)
        nc.vector.memset(V[:, :, 0:8, 64:66], 1.0)
        nc.vector.memset(V[:, :, 8, :], 0.0)

        ph1 = ExitStack()
        xpool = ph1.enter_context(tc.tile_pool(name="x", bufs=8))
        wpool = ph1.enter_context(tc.tile_pool(name="w", bufs=2))
        ph2b = ExitStack()
        bpool = ph2b.enter_context(tc.tile_pool(name="b", bufs=2))
        oupool = ph2b.enter_context(tc.tile_pool(name="ou", bufs=1))
        dpool = ph2b.enter_context(tc.tile_pool(name="dscr", bufs=4, space="DRAM"))

        # E[h][si][f] -> AP (bf16 view) for the PV matmuls
        E: dict = {h: {} for h in range(8)}
        expc = [0]  # exp tile counter for the engine split

        def emit_exp(h, si, f, ps):
            c = expc[0]
            expc[0] += 1
            use_dve = ((c + 1) * DVE_NUM) // DVE_DEN > (c * DVE_NUM) // DVE_DEN
            if use_dve:
                e = epool.tile([128, 512], I16, tag="e", name=f"e_{h}_{si}_{f}")
                nc.vector.tensor_scalar(
                    e[:], ps[:], SCH_A, SCH_B, ALU.mult, ALU.add
                )
                E[h].setdefault(si, {})[f] = e.bitcast(BF16)
            else:
                e = epool.tile([128, 512], BF16, tag="e", name=f"e_{h}_{si}_{f}")
                nc.scalar.activation(e[:], ps[:], AF.Exp)
                E[h].setdefault(si, {})[f] = e[:]

        def emit_qk(p, si, halves=(0, 1)):
            """Row-tiled pair: head 2p on PE rows 0-63, head 2p+1 on 64-127."""
            for hf in halves:
                for f in range(hf * nf // 2, (hf + 1) * nf // 2):
                    psA = pss.tile([128, 512], F32, tag="s", name=f"sA_{p}_{si}_{f}")
                    psB = pss.tile([128, 512], F32, tag="s", name=f"sB_{p}_{si}_{f}")
                    nc.tensor.matmul(
                        psA[:], lhsT=KT[ds(0, 64), p, ts(si, 128)],
                        rhs=QT[ds(0, 64), p, ts(f, 512)], start=True, stop=True,
                    )
                    nc.tensor.matmul(
                        psB[:], lhsT=KT[ds(64, 64), p, ts(si, 128)],
                        rhs=QT[ds(64, 64), p, ts(f, 512)], start=True, stop=True,
                    )
                    emit_exp(2 * p, si, f, psA)
                    emit_exp(2 * p + 1, si, f, psB)

        def emit_pv(h, si, pos):
            for f in range(nf):
                nc.tensor.matmul(
                    pos[f][:, :],
                    lhsT=Vt[:, si, ds(h * 66, 128)],
                    rhs=E[h][si].pop(f),
                    start=(si == 0),
                    stop=(si == nsi - 1),
                )

        def pv_finish(h, pos):
            """Per f-block: copy O_unnorm^T + denom out of PSUM (ScalarE,
            frees the accumulator bank), run the denominator chain (DMA
            reshape -> DVE reciprocal -> DMA partition-broadcast) and
            normalize on GpSimd.  f-granular so the last head's chain
            pipelines with the output projection."""
            hp, hh = h // 2, (h % 2) * 64
            ou = oupool.tile([65, s], F32, tag="ou", name=f"ou_{h}")
            for f in range(nf):
                nc.scalar.copy(ou[:, ts(f, 512)], pos[f][0:65, :])
                dscr = dpool.tile([1, 512], F32, tag="dscr", name=f"dscr_{h}_{f}")
                nc.sync.dma_start(dscr[:], ou[ds(64, 1), ts(f, 512)])
                d16 = bpool.tile([16, 32], F32, tag="d16", name=f"d16_{h}_{f}")
                nc.sync.dma_start(
                    d16[:], dscr[:].rearrange("one (p c) -> (one p) c", p=16)
                )
                r16 = bpool.tile([16, 32], F32, tag="r16", name=f"r16_{h}_{f}")
                nc.vector.reciprocal(r16[:], d16[:])
                dsc2 = dpool.tile([1, 512], F32, tag="dsc2", name=f"dsc2_{h}_{f}")
                nc.sync.dma_start(
                    dsc2[:].rearrange("one (p c) -> (one p) c", p=16), r16[:]
                )
                bsb = bpool.tile([64, 512], F32, tag="bsb", name=f"bsb_{h}_{f}")
                nc.sync.dma_start(bsb[:], dsc2[:].to_broadcast((64, 512)))
                nc.gpsimd.tensor_tensor(
                    O[ds(hh, 64), hp, ts(f, 512)],
                    ou[0:64, ts(f, 512)],
                    bsb[:],
                    ALU.mult,
                )

        # ---- phase A: Q'/K' projections -------------------------------
        for xdram, wdram, dst, bias in ((qT, wq, QT, BQ), (kT, wk, KT, None)):
            wt = wpool.tile([128, 8, 512], BF16, tag="w")
            for ki in range(8):
                nc.sync.dma_start(wt[:, ki, :], wdram[ds(ki * 128, 128), :])
            for fp in range(nfp):
                xts = []
                for ki in range(8):
                    xt = xpool.tile([128, 1024], BF16, tag="x")
                    nc.sync.dma_start(
                        xt[:], xdram[ds(ki * 128, 128), ds(fp * 1024, 1024)]
                    )
                    xts.append(xt)
                for pc in range(4):
                    for half in range(2):
                        ps = pspool.tile([128, 512], F32, tag="ps")
                        for ki in range(8):
                            nc.tensor.matmul(
                                ps[:],
                                lhsT=wt[:, ki, ts(pc, 128)],
                                rhs=xts[ki][:, ts(half, 512)],
                                start=(ki == 0),
                                stop=(ki == 7),
                            )
                        f = 2 * fp + half
                        if bias is not None:
                            nc.vector.tensor_scalar_add(
                                dst[:, pc, ts(f, 512)], ps[:],
                                bias[:, pc : pc + 1],
                            )
                        else:
                            nc.scalar.copy(dst[:, pc, ts(f, 512)], ps[:])

        # ---- w0: V projection + pair-0 QK/exp + head-0 PV -------------
        wtv = wpool.tile([128, 8, 512], BF16, tag="w")
        for ki in range(8):
            nc.sync.dma_start(wtv[:, ki, :], wv[ds(ki * 128, 128), :])
        pos_cur = [
            pspool.tile([128, 512], F32, tag="ps", name=f"pos_0_{i}")
            for i in range(nf)
        ]
        xv = None
        for si in range(nsi):
            fv, sj = si // 8, si % 8
            if sj == 0:
                xv = []
                for ki in range(8):
                    xt = xpool.tile([128, 1024], BF16, tag="x")
                    nc.sync.dma_start(
                        xt[:], vT[ds(ki * 128, 128), ds(fv * 1024, 1024)]
                    )
                    xv.append(xt)
            vps = pss.tile([128, 512], F32, tag="s", name=f"vps_{si}")
            for ki in range(8):
                nc.tensor.matmul(
                    vps[:],
                    lhsT=xv[ki][:, ts(sj, 128)],
                    rhs=wtv[:, ki, :],
                    start=(ki == 0),
                    stop=(ki == 7),
                )
            nc.scalar.copy(
                V[:, si, 0:8, 0:64], vps[:].rearrange("p (h d) -> p h d", h=8)
            )
            emit_qk(0, si, halves=(0,))
            if si > 0:
                emit_pv(0, si - 1, pos_cur)
            emit_qk(0, si, halves=(1,))
        emit_pv(0, nsi - 1, pos_cur)
        pv_finish(0, pos_cur)

        for ki in range(4):
            nc.sync.dma_start(WO[:, ki, :], wo[ds(ki * 128, 128), :])

        # ---- w1..w7: drain heads 1..7, produce pairs 1..3 at half rate
        qk_chunks = [(p, si) for p in range(1, 4) for si in range(nsi)]
        qi = 0
        for hd in range(1, 8):
            pos_cur = [
                pspool.tile([128, 512], F32, tag="ps", name=f"pos_{hd}_{i}")
                for i in range(nf)
            ]
            for si in range(nsi):
                chunk = qk_chunks[qi] if (si % 2 == 0 and qi < len(qk_chunks)) else None
                if chunk is not None:
                    emit_qk(*chunk, halves=(0,))
                    qi += 1
                emit_pv(hd, si, pos_cur)
                if chunk is not None:
                    emit_qk(*chunk, halves=(1,))
            pv_finish(hd, pos_cur)
        ph2b.close()
        ph1.close()

        # ---- phase C: output projection (partial over this core's heads)
        opool = ctx.enter_context(tc.tile_pool(name="ostage", bufs=3))
        outr = outT.rearrange("(o p) n -> p o n", p=128)
        for f in range(nf):
            for pe in range(8):
                ps = pspool.tile([128, 512], F32, tag="ps")
                for ki in range(4):
                    nc.tensor.matmul(
                        ps[:],
                        lhsT=WO[:, ki, ts(pe, 128)],
                        rhs=O[:, ki, ts(f, 512)],
                        start=(ki == 0),
                        stop=(ki == 3),
                    )
                ot = opool.tile([128, 512], F32, tag="ot")
                if (pe + f) % 2 == 0:
                    nc.vector.tensor_copy(ot[:], ps[:])
                else:
                    nc.scalar.copy(ot[:], ps[:])
                nc.sync.dma_start(outr[:, pe, ts(f, 512)], ot[:])

    nc.compile()
    return nc


_NC_CACHE: dict = {}


def get_nc(s: int = S):
    if s not in _NC_CACHE:
        _NC_CACHE[s] = build_nc(s)
    return _NC_CACHE[s]


def _prep_in_maps(q, k, v, Wq, bq, Wk, Wv, Wo):
    """Host-side shard prep: per-core input dicts (cheap numpy reshapes)."""
    f32 = np.float32
    scale = 1.0 / np.sqrt(DK)
    xT = {}
    for b in range(B):
        xT[b] = (
            np.ascontiguousarray(q[b].T).astype(BF16NP),
            np.ascontiguousarray(k[b].T).astype(BF16NP),
            np.ascontiguousarray(v[b].T).astype(BF16NP),
        )
    per_g = {}
    for g in range(2):
        F = slice(512 * g, 512 * g + 512)
        per_g[g] = dict(
            wq=np.ascontiguousarray(Wq[F].T * scale).astype(BF16NP),
            wk=np.ascontiguousarray(Wk[F].T).astype(BF16NP),
            wv=np.ascontiguousarray(Wv[F].T).astype(BF16NP),
            wo=np.ascontiguousarray(Wo[:, F].T).astype(BF16NP),
            bq=np.ascontiguousarray(
                (bq[F] * scale).reshape(4, 128).T, dtype=f32
            ),
        )
    in_maps = []
    for c in range(N_CORES):
        b, g = c // 2, c % 2
        qb, kb, vb = xT[b]
        in_maps.append(dict(qT=qb, kT=kb, vT=vb, **per_g[g]))
    return in_maps


def kernel(q, k, v, Wq, bq, Wk, bk, Wv, bv, Wo, bo):
    q, k, v = (np.asarray(x, np.float32) for x in (q, k, v))
    Wq, bq, Wk, bk = (np.asarray(x, np.float32) for x in (Wq, bq, Wk, bk))
    Wv, bv, Wo, bo = (np.asarray(x, np.float32) for x in (Wv, bv, Wo, bo))

    nc = get_nc(S)
    in_maps = _prep_in_maps(q, k, v, Wq, bq, Wk, Wv, Wo)
    res = run_bass_kernel_spmd(nc, in_maps, core_ids=list(range(N_CORES)))

    # bk drops out of softmax; bv folds into an effective output bias.
    bo_eff = (
        bo.astype(np.float64) + Wo.astype(np.float64) @ bv.astype(np.float64)
    ).astype(np.float32)
    out = np.empty((B, S, D), np.float32)
    for b in range(B):
        acc = res.results[2 * b]["outT"] + res.results[2 * b + 1]["outT"]
        out[b] = acc.T + bo_eff
    return out
